# revision 1
# baseline (speedup 1.0000x reference)
"""Trainium2 Bass kernel for CustomAttention (dense transformer block).

Full inputs -> full output. Tensor-parallel over heads across 8 NeuronCores:
core c owns heads [4c, 4c+4) i.e. projection columns [512c, 512c+512).
Each core computes q/k/v projections for its heads (RoPE applied on-chip),
causal attention (softmax without max-subtraction; scores bounded ~19), and
a partial output projection over its 512-wide slice of the contraction dim.
The host sums the 8 partials.

All operands are bf16 (1 col/cycle matmul streaming + FWL weight loads);
PSUM accumulation is fp32. q/k/v stay SBUF-resident between phases (no DRAM
spill). DMA is split across the two HW DGE rings: x-tile streams + output
writes on qSync, weight/table bulk loads + rope swaps on qAct. Attention
exp runs on score PAIRS ([128,1024] from 2 PSUM banks) to halve activation
overhead; the softmax denominator is a vector bf16 add-tree plus one
ones-matmul per query block; the output projection is interleaved at
query-block granularity to keep TensorE fed during exp-bound stretches.
"""

import numpy as np

import concourse.bass as bass
import concourse.tile as tile
from concourse import bacc, mybir
from concourse.bass_utils import run_bass_kernel_spmd

F32 = mybir.dt.float32
BF16 = mybir.dt.bfloat16
EXPFN = mybir.ActivationFunctionType.Exp

D = 4096          # model dim
H = 32            # heads (total)
HD = 128          # head dim
NCORES = 8
HPC = H // NCORES  # heads per core = 4
MS = HPC * HD      # per-core projection slice = 512
B = 2
S = 2048
T = B * S         # 4096 tokens
DT = D // 128     # 32 contraction tiles
SCALE = HD ** -0.5

_compiled = {}


def _build():
    nc = bacc.Bacc("TRN2", target_bir_lowering=False, debug=False,
                   num_devices=NCORES)

    xT_d = nc.dram_tensor("xT", [D, T], BF16, kind="ExternalInput").ap()
    wqT_d = nc.dram_tensor("wqT", [D, MS], BF16, kind="ExternalInput").ap()
    wkT_d = nc.dram_tensor("wkT", [D, MS], BF16, kind="ExternalInput").ap()
    wvT_d = nc.dram_tensor("wvT", [D, MS], BF16, kind="ExternalInput").ap()
    woT_d = nc.dram_tensor("woT", [MS, D], BF16, kind="ExternalInput").ap()
    cos_d = nc.dram_tensor("cosT", [HD, S], BF16, kind="ExternalInput").ap()
    ssin_d = nc.dram_tensor("ssinT", [HD, S], BF16, kind="ExternalInput").ap()
    hmask_d = nc.dram_tensor("hmask", [128, 896], F32, kind="ExternalInput").ap()
    out_d = nc.dram_tensor("outp", [T, D], F32, kind="ExternalOutput").ap()

    with tile.TileContext(nc) as tc:
        _emit(nc, tc, xT_d, wqT_d, wkT_d, wvT_d, woT_d, cos_d, ssin_d,
              hmask_d, out_d)

    nc.compile()
    return nc


def _emit(nc, tc, xT_d, wqT_d, wkT_d, wvT_d, woT_d, cos_d, ssin_d,
          hmask_d, out_d):
    from contextlib import ExitStack

    TB = 512                 # token block for projections
    NTB = T // TB            # 8
    IT = 512                 # i-tile (query) width in attention
    NIT = S // IT            # 4 per batch
    XC = 4                   # dt per x-load chunk

    def load_w(w_sb, w_d, g):
        sl = slice(g * 4 * 128, (g + 1) * 4 * 128)
        nc.scalar.dma_start(
            w_sb[:, g * 4:(g + 1) * 4, :],
            w_d[sl, :].rearrange("(dt p) m -> p dt m", p=128))

    with ExitStack() as ctx:
        const_pool = ctx.enter_context(tc.tile_pool(name="const", bufs=1))
        hmask_sb = const_pool.tile([128, 896], F32)
        ones_sb = const_pool.tile([128, 128], BF16)
        nc.vector.memset(ones_sb[:], 1.0)

        # SBUF-resident q/k for the whole kernel.
        qk_pool = ctx.enter_context(tc.tile_pool(name="qkres", bufs=1))
        q_sb = qk_pool.tile([128, HPC, T], BF16)
        k_sb = qk_pool.tile([128, HPC, T], BF16)

        # wv lives from A1 (prefetch) through A2.
        wv_pool = ctx.enter_context(tc.tile_pool(name="wv", bufs=1))
        wv_sb = wv_pool.tile([128, DT, MS], BF16)

        # ================= phase A1: q/k projections + rope ================
        with ExitStack() as p1:
            wq_pool = p1.enter_context(tc.tile_pool(name="wqk", bufs=1))
            tbl_pool = p1.enter_context(tc.tile_pool(name="tbl", bufs=1))
            x_pool = p1.enter_context(tc.tile_pool(name="x1", bufs=3))
            ps_pool = p1.enter_context(
                tc.tile_pool(name="psA", bufs=8, space="PSUM"))
            rp_pool = p1.enter_context(tc.tile_pool(name="rope", bufs=2))

            cos_sb = tbl_pool.tile([HD, S], BF16)
            ssin_sb = tbl_pool.tile([HD, S], BF16)
            wq_sb = wq_pool.tile([128, DT, MS], BF16)
            wk_sb = wq_pool.tile([128, DT, MS], BF16)

            # qAct ring order: single-dt first chunks for fastest start,
            # then tables, remaining chunks, then the wv prefetch.
            for dt in range(4):
                nc.scalar.dma_start(
                    wq_sb[:, dt, :], wqT_d[dt * 128:(dt + 1) * 128, :])
                nc.scalar.dma_start(
                    wk_sb[:, dt, :], wkT_d[dt * 128:(dt + 1) * 128, :])
            nc.scalar.dma_start(cos_sb[:], cos_d[:])
            nc.scalar.dma_start(ssin_sb[:], ssin_d[:])
            nc.scalar.dma_start(hmask_sb[:], hmask_d[:])
            for g in range(1, 8):
                load_w(wq_sb, wqT_d, g)
                load_w(wk_sb, wkT_d, g)
            for g in range(8):
                load_w(wv_sb, wvT_d, g)

            for tb in range(NTB):
                tsl = bass.ts(tb, TB)
                psl = slice((tb * TB) % S, (tb * TB) % S + TB)
                for w_sb, dst_sb in ((wq_sb, q_sb), (wk_sb, k_sb)):
                    pss = [ps_pool.tile([128, TB], F32, tag="ps",
                                        name=f"psA_{_g}")
                           for _g in range(HPC)]
                    for g in range(DT // XC):
                        xc = x_pool.tile([128, XC, TB], BF16, tag="x")
                        nc.sync.dma_start(
                            xc[:],
                            xT_d[g * XC * 128:(g + 1) * XC * 128, tsl]
                            .rearrange("(dt p) t -> p dt t", p=128))
                        for j in range(XC):
                            dt = g * XC + j
                            for mt in range(HPC):
                                nc.tensor.matmul(
                                    pss[mt][:],
                                    w_sb[:, dt, mt * 128:(mt + 1) * 128],
                                    xc[:, j, :],
                                    start=(dt == 0), stop=(dt == DT - 1))
                    for mt in range(HPC):
                        raw = rp_pool.tile([128, TB], F32, tag="raw")
                        nc.scalar.copy(raw[:], pss[mt][:])
                        # rotate-half operand: partitions swapped by 64
                        sw = rp_pool.tile([128, TB], F32, tag="sw")
                        nc.scalar.dma_start(sw[0:64, :], raw[64:128, :])
                        nc.scalar.dma_start(sw[64:128, :], raw[0:64, :])
                        qc = rp_pool.tile([128, TB], F32, tag="qc")
                        nc.vector.tensor_mul(qc[:], raw[:], cos_sb[:, psl])
                        qs = rp_pool.tile([128, TB], F32, tag="qs")
                        nc.vector.tensor_mul(qs[:], sw[:], ssin_sb[:, psl])
                        nc.vector.tensor_add(
                            dst_sb[:, mt, tsl], qc[:], qs[:])

        # ================= phase A2 + B share v_sb / wo residency ==========
        with ExitStack() as pAB:
            vres_pool = pAB.enter_context(tc.tile_pool(name="vres", bufs=1))
            wo_pool = pAB.enter_context(tc.tile_pool(name="wo", bufs=1))
            v_sb = vres_pool.tile([128, T // 128, MS], BF16)
            wo_sb = wo_pool.tile([128, HPC, D], BF16)

            # ---- A2: v projection (x-stationary) --------------------------
            with ExitStack() as p2:
                x2_pool = p2.enter_context(tc.tile_pool(name="x2", bufs=3))
                ps2_pool = p2.enter_context(
                    tc.tile_pool(name="psA2", bufs=8, space="PSUM"))

                for g in range(4):
                    nc.scalar.dma_start(
                        wo_sb[:, g, :], woT_d[g * 128:(g + 1) * 128, :])

                for tb in range(NTB):
                    tsl = bass.ts(tb, TB)
                    pss = [ps2_pool.tile([128, MS], F32, tag="ps",
                                         name=f"psA2_{_g}")
                           for _g in range(TB // 128)]
                    for g in range(DT // XC):
                        xc = x2_pool.tile([128, XC, TB], BF16, tag="x")
                        nc.sync.dma_start(
                            xc[:],
                            xT_d[g * XC * 128:(g + 1) * XC * 128, tsl]
                            .rearrange("(dt p) t -> p dt t", p=128))
                        for j in range(XC):
                            dt = g * XC + j
                            for tt in range(TB // 128):
                                nc.tensor.matmul(
                                    pss[tt][:],
                                    xc[:, j, tt * 128:(tt + 1) * 128],
                                    wv_sb[:, dt, :],
                                    start=(dt == 0), stop=(dt == DT - 1))
                    for tt in range(TB // 128):
                        nc.vector.tensor_copy(
                            v_sb[:, tb * (TB // 128) + tt, :], pss[tt][:])

            # ---- B: attention + output projection -------------------------
            with ExitStack() as p3:
                ctx_pool = p3.enter_context(tc.tile_pool(name="ctx", bufs=6))
                e_pool = p3.enter_context(tc.tile_pool(name="expt", bufs=4))
                rs_pool = p3.enter_context(tc.tile_pool(name="rsum", bufs=6))
                n_pool = p3.enter_context(tc.tile_pool(name="norm", bufs=2))
                o_pool = p3.enter_context(tc.tile_pool(name="osb", bufs=3))
                s_ps_pool = p3.enter_context(
                    tc.tile_pool(name="sps", bufs=2, space="PSUM"))
                a_ps_pool = p3.enter_context(
                    tc.tile_pool(name="aps", bufs=2, space="PSUM"))
                o_ps_pool = p3.enter_context(
                    tc.tile_pool(name="ops", bufs=2, space="PSUM"))

                for b in range(B):
                    for i in range(NIT):
                        ctx_i = []      # per-head [128, IT] ctx for this i
                        for h in range(HPC):
                            qt = q_sb[:, h,
                                      b * S + i * IT: b * S + (i + 1) * IT]
                            ctx_ps = a_ps_pool.tile([128, IT], F32,
                                                    tag="ctxps")
                            njt = (i + 1) * IT // 128
                            npair = njt // 2
                            # binary reduction ladder for the denominator:
                            # combine eagerly so at most one live tile per
                            # level (bounded pool pressure, no deadlock)
                            levels = {}

                            def rs_push(t, lvl=0):
                                while lvl in levels:
                                    prev = levels.pop(lvl)
                                    acc = rs_pool.tile([128, IT], BF16,
                                                       tag="ps2")
                                    nc.vector.tensor_add(acc[:], prev[:],
                                                         t[:])
                                    t = acc
                                    lvl += 1
                                levels[lvl] = t
                            for p in range(npair):
                                jt0 = 2 * p
                                s_ps = s_ps_pool.tile([128, 1024], F32,
                                                      tag="sps")
                                for half in range(2):
                                    jt = jt0 + half
                                    nc.tensor.matmul(
                                        s_ps[:, half * 512:(half + 1) * 512],
                                        k_sb[:, h, b * S + jt * 128:
                                             b * S + (jt + 1) * 128],
                                        qt, start=True, stop=True)
                                et = e_pool.tile([128, 2 * IT], BF16,
                                                 tag="et")
                                doff0 = jt0 * 128 - i * IT
                                if doff0 + 128 < 0:   # fully off-diagonal
                                    nc.scalar.activation(
                                        et[:], s_ps[:], EXPFN, scale=SCALE)
                                else:
                                    ef = e_pool.tile([128, 1024], F32,
                                                     tag="ef", bufs=2)
                                    nc.scalar.activation(ef[:], s_ps[:],
                                                         EXPFN, scale=SCALE)
                                    for half in range(2):
                                        doff = doff0 + half * 128
                                        nc.vector.tensor_mul(
                                            et[:, half * IT:
                                               (half + 1) * IT],
                                            ef[:, half * 512:
                                               (half + 1) * 512],
                                            hmask_sb[:, 384 - doff:
                                                     896 - doff])
                                for half in range(2):
                                    jt = jt0 + half
                                    nc.tensor.matmul(
                                        ctx_ps[:],
                                        v_sb[:, b * (S // 128) + jt,
                                             h * 128:(h + 1) * 128],
                                        et[:, half * IT:(half + 1) * IT],
                                        start=(jt == 0),
                                        stop=(jt == njt - 1))
                                psum = rs_pool.tile([128, IT], BF16,
                                                    tag="ps2")
                                nc.vector.tensor_add(
                                    psum[:], et[:, 0:IT], et[:, IT:2 * IT])
                                rs_push(psum)
                            # fold remaining ladder levels
                            rem = [levels[l] for l in sorted(levels)]
                            total = rem[0]
                            for t in rem[1:]:
                                acc = rs_pool.tile([128, IT], BF16,
                                                   tag="ps2")
                                nc.vector.tensor_add(acc[:], total[:], t[:])
                                total = acc
                            rs_ps = s_ps_pool.tile([128, 1024], F32,
                                                   tag="sps")
                            nc.tensor.matmul(rs_ps[:, 0:512], ones_sb[:],
                                             total[:],
                                             start=True, stop=True)
                            recip = n_pool.tile([128, IT], F32, tag="recip")
                            nc.vector.reciprocal_approx_fast(
                                recip[:], rs_ps[:, 0:512])
                            ctx_h = ctx_pool.tile([128, IT], BF16, tag="ctx")
                            nc.vector.tensor_mul(ctx_h[:], ctx_ps[:],
                                                 recip[:])
                            ctx_i.append(ctx_h)

                        # output projection for this query block
                        for tt in range(IT // 128):
                            for et_i in range(D // 512):
                                o_ps = o_ps_pool.tile([128, 512], F32,
                                                      tag="ops")
                                for h in range(HPC):
                                    nc.tensor.matmul(
                                        o_ps[:],
                                        ctx_i[h][:, tt * 128:(tt + 1) * 128],
                                        wo_sb[:, h,
                                              et_i * 512:(et_i + 1) * 512],
                                        start=(h == 0), stop=(h == HPC - 1))
                                osb = o_pool.tile([128, 512], F32, tag="osb")
                                nc.vector.tensor_copy(osb[:], o_ps[:])
                                row = b * S + i * IT + tt * 128
                                nc.sync.dma_start(
                                    out_d[row:row + 128,
                                          et_i * 512:(et_i + 1) * 512],
                                    osb[:])


def _host_prep(x, Wq, Wk, Wv, Wo):
    import ml_dtypes
    bf16 = ml_dtypes.bfloat16

    x = np.asarray(x, dtype=np.float32)
    xT = np.ascontiguousarray(x.reshape(T, D).T.astype(bf16))     # [D, T]

    # per-core column slices of W.T  -> [ncores][D, MS]
    def col_shards(W):
        WT = np.asarray(W, dtype=np.float32).T.reshape(D, NCORES, MS)
        return np.ascontiguousarray(WT.transpose(1, 0, 2).astype(bf16))
    wqT = col_shards(Wq)
    wkT = col_shards(Wk)
    wvT = col_shards(Wv)
    # per-core row slices of Wo.T -> [ncores][MS, D]
    woT = np.ascontiguousarray(
        np.asarray(Wo, dtype=np.float32).T.reshape(NCORES, MS, D).astype(bf16))

    # rope tables in [hd, s] layout
    inv = (1.0 / (10000.0 ** (np.arange(0, HD, 2, dtype=np.float32) / HD))
           ).astype(np.float32)
    t = np.arange(S, dtype=np.float32)
    freqs = np.outer(t, inv).astype(np.float32)                # [S, 64]
    cos = np.cos(freqs).T                                      # [64, S]
    sin = np.sin(freqs).T
    cosT = np.ascontiguousarray(
        np.concatenate([cos, cos], axis=0).astype(bf16))       # [128, S]
    ssinT = np.ascontiguousarray(
        np.concatenate([-sin, sin], axis=0).astype(bf16))

    # causal mask table: hmask[dj, y] = 1 if dj <= y - 384
    dj = np.arange(128)[:, None]
    y = np.arange(896)[None, :]
    hmask = (dj <= y - 384).astype(np.float32)

    return xT, wqT, wkT, wvT, woT, cosT, ssinT, hmask


def kernel(x, mask, Wq, Wk, Wv, Wo, _trace=False):
    del mask  # causal mask is hardcoded (tril), matching the reference
    xT, wqT, wkT, wvT, woT, cosT, ssinT, hmask = _host_prep(x, Wq, Wk, Wv, Wo)

    if "nc" not in _compiled:
        _compiled["nc"] = _build()
    nc = _compiled["nc"]

    in_maps = []
    for c in range(NCORES):
        in_maps.append({
            "xT": xT,
            "wqT": wqT[c],
            "wkT": wkT[c],
            "wvT": wvT[c],
            "woT": woT[c],
            "cosT": cosT,
            "ssinT": ssinT,
            "hmask": hmask,
        })

    res = run_bass_kernel_spmd(nc, in_maps, core_ids=list(range(NCORES)),
                               trace=_trace)

    acc = res.results[0]["outp"].astype(np.float64)
    for c in range(1, NCORES):
        acc += res.results[c]["outp"]
    out = acc.astype(np.float32).reshape(B, S, D)
    if _trace:
        kernel.last_exec_time_ns = res.exec_time_ns
        kernel.last_results = res
    return out



# revision 8
# speedup vs baseline: 1.1243x; 1.1243x over previous
"""Trainium2 Bass kernel for CustomAttention (dense transformer block).

Full inputs -> full output. Tensor-parallel over heads across 8 NeuronCores:
core c owns heads [4c, 4c+4) i.e. projection columns [512c, 512c+512).
Each core computes q/k/v projections for its heads (RoPE applied on-chip),
causal attention (softmax without max-subtraction; scores bounded ~19), and
a partial output projection over its 512-wide slice of the contraction dim.
The host sums the 8 partials.

All operands are bf16 (1 col/cycle matmul streaming + FWL weight loads);
PSUM accumulation is fp32. q/k/v stay SBUF-resident between phases (no DRAM
spill). DMA is split across the two HW DGE rings: x-tile streams + output
writes on qSync, weight/table bulk loads + rope swaps on qAct. The x tile
pool and projection PSUM pool are shared between the q/k and v projection
phases so the v matmuls start while the last rope tail drains. wv loads are
spread across the q/k loop so rope swap DMAs don't queue behind bulk loads.

Attention exp runs on score PAIRS ([128,1024] from 2 PSUM banks) to halve
activation overhead; the softmax denominator is a vector bf16 add-tree plus
one ones-matmul per query block. The output projection for query block n is
software-pipelined into the attention of block n+1: its matmuls fill tensor
bubbles left by the scores->exp->pv dependency chain, and its PSUM results
drain through the scalar engine (bf16 cast) into [128, 4096] row tiles
written with one DMA per 128 tokens, keeping the vector engine off the
tensor critical path.
"""

import numpy as np

import concourse.bass as bass
import concourse.tile as tile
from concourse import bacc, mybir
from concourse.bass_utils import run_bass_kernel_spmd

F32 = mybir.dt.float32
BF16 = mybir.dt.bfloat16
EXPFN = mybir.ActivationFunctionType.Exp

D = 4096          # model dim
H = 32            # heads (total)
HD = 128          # head dim
NCORES = 8
HPC = H // NCORES  # heads per core = 4
MS = HPC * HD      # per-core projection slice = 512
B = 2
S = 2048
T = B * S         # 4096 tokens
DT = D // 128     # 32 contraction tiles
SCALE = HD ** -0.5

_compiled = {}


def _build():
    nc = bacc.Bacc("TRN2", target_bir_lowering=False, debug=False,
                   num_devices=NCORES)

    xT_d = nc.dram_tensor("xT", [D, T], BF16, kind="ExternalInput").ap()
    wqT_d = nc.dram_tensor("wqT", [D, MS], BF16, kind="ExternalInput").ap()
    wkT_d = nc.dram_tensor("wkT", [D, MS], BF16, kind="ExternalInput").ap()
    wvT_d = nc.dram_tensor("wvT", [D, MS], BF16, kind="ExternalInput").ap()
    woT_d = nc.dram_tensor("woT", [MS, D], BF16, kind="ExternalInput").ap()
    cos_d = nc.dram_tensor("cosT", [HD, S], BF16, kind="ExternalInput").ap()
    ssin_d = nc.dram_tensor("ssinT", [HD, S], BF16, kind="ExternalInput").ap()
    hmask_d = nc.dram_tensor("hmask", [128, 896], BF16, kind="ExternalInput").ap()
    out_d = nc.dram_tensor("outp", [T, D], BF16, kind="ExternalOutput").ap()

    with tile.TileContext(nc) as tc:
        _emit(nc, tc, xT_d, wqT_d, wkT_d, wvT_d, woT_d, cos_d, ssin_d,
              hmask_d, out_d)

    nc.compile()
    return nc


def _emit(nc, tc, xT_d, wqT_d, wkT_d, wvT_d, woT_d, cos_d, ssin_d,
          hmask_d, out_d):
    from contextlib import ExitStack

    TB = 512                 # token block for projections
    NTB = T // TB            # 8
    IT = 512                 # i-tile (query) width in attention
    NIT = S // IT            # 4 per batch
    XC = 4                   # dt per x-load chunk

    def load_w(w_sb, w_d, g):
        sl = slice(g * 4 * 128, (g + 1) * 4 * 128)
        nc.scalar.dma_start(
            w_sb[:, g * 4:(g + 1) * 4, :],
            w_d[sl, :].rearrange("(dt p) m -> p dt m", p=128))

    with ExitStack() as ctx:
        const_pool = ctx.enter_context(tc.tile_pool(name="const", bufs=1))
        hmask_sb = const_pool.tile([128, 896], BF16)
        ones_sb = const_pool.tile([128, 128], BF16)
        nc.vector.memset(ones_sb[:], 1.0)

        # SBUF-resident q/k/v for the whole kernel.
        qk_pool = ctx.enter_context(tc.tile_pool(name="qkres", bufs=1))
        q_sb = qk_pool.tile([128, HPC, T], BF16)
        k_sb = qk_pool.tile([128, HPC, T], BF16)
        v_sb = qk_pool.tile([128, T // 128, MS], BF16)

        # ============ phases A0 (v) and A1 (q/k + rope) ====================
        # v first: its weight pool then frees into the q/k weight pool, and
        # the shared x/PSUM pools let the phases overlap at the boundary.
        with ExitStack() as pa:
            x_pool = pa.enter_context(tc.tile_pool(name="xs", bufs=3))
            ps_pool = pa.enter_context(
                tc.tile_pool(name="psA", bufs=8, space="PSUM"))

            with ExitStack() as p0:
                wv_pool = p0.enter_context(tc.tile_pool(name="wv", bufs=1))
                wv_sb = wv_pool.tile([128, DT, MS], BF16)
                for g in range(8):
                    load_w(wv_sb, wvT_d, g)

                for tb in range(NTB):
                    tsl = bass.ts(tb, TB)
                    pss = [ps_pool.tile([128, MS], F32, tag="ps",
                                        name=f"psA0_{_g}")
                           for _g in range(TB // 128)]
                    for g in range(DT // XC):
                        xc = x_pool.tile([128, XC, TB], BF16, tag="x")
                        nc.sync.dma_start(
                            xc[:],
                            xT_d[g * XC * 128:(g + 1) * XC * 128, tsl]
                            .rearrange("(dt p) t -> p dt t", p=128))
                        for j in range(XC):
                            dt = g * XC + j
                            for tt in range(TB // 128):
                                nc.tensor.matmul(
                                    pss[tt][:],
                                    xc[:, j, tt * 128:(tt + 1) * 128],
                                    wv_sb[:, dt, :],
                                    start=(dt == 0), stop=(dt == DT - 1))
                    for tt in range(TB // 128):
                        nc.vector.tensor_copy(
                            v_sb[:, tb * (TB // 128) + tt, :], pss[tt][:])

            # ---- A1: q/k projections + rope (wv space -> wq/wk) -----------
            with ExitStack() as p1:
                wq_pool = p1.enter_context(tc.tile_pool(name="wqk", bufs=1))
                tbl_pool = p1.enter_context(tc.tile_pool(name="tbl", bufs=1))
                rp_pool = p1.enter_context(tc.tile_pool(name="rope", bufs=2))

                cos_sb = tbl_pool.tile([HD, S], BF16)
                ssin_sb = tbl_pool.tile([HD, S], BF16)
                wq_sb = wq_pool.tile([128, DT, MS], BF16)
                wk_sb = wq_pool.tile([128, DT, MS], BF16)

                # qAct ring order: single-dt first chunks for fastest start,
                # then tables, then the remaining chunks.
                for dt in range(4):
                    nc.scalar.dma_start(
                        wq_sb[:, dt, :], wqT_d[dt * 128:(dt + 1) * 128, :])
                    nc.scalar.dma_start(
                        wk_sb[:, dt, :], wkT_d[dt * 128:(dt + 1) * 128, :])
                nc.scalar.dma_start(cos_sb[:], cos_d[:])
                nc.scalar.dma_start(ssin_sb[:], ssin_d[:])
                for g in range(1, 8):
                    load_w(wq_sb, wqT_d, g)
                    load_w(wk_sb, wkT_d, g)

                for tb in range(NTB):
                    tsl = bass.ts(tb, TB)
                    psl = slice((tb * TB) % S, (tb * TB) % S + TB)
                    for w_sb, dst_sb in ((wq_sb, q_sb), (wk_sb, k_sb)):
                        pss = [ps_pool.tile([128, TB], F32, tag="ps",
                                            name=f"psA_{_g}")
                               for _g in range(HPC)]
                        for g in range(DT // XC):
                            xc = x_pool.tile([128, XC, TB], BF16, tag="x")
                            nc.sync.dma_start(
                                xc[:],
                                xT_d[g * XC * 128:(g + 1) * XC * 128, tsl]
                                .rearrange("(dt p) t -> p dt t", p=128))
                            for j in range(XC):
                                dt = g * XC + j
                                for mt in range(HPC):
                                    nc.tensor.matmul(
                                        pss[mt][:],
                                        w_sb[:, dt, mt * 128:(mt + 1) * 128],
                                        xc[:, j, :],
                                        start=(dt == 0), stop=(dt == DT - 1))
                        for mt in range(HPC):
                            raw = rp_pool.tile([128, TB], F32, tag="raw")
                            nc.scalar.copy(raw[:], pss[mt][:])
                            # rotate-half operand: partitions swapped by 64
                            sw = rp_pool.tile([128, TB], F32, tag="sw")
                            nc.scalar.dma_start(sw[0:64, :], raw[64:128, :])
                            nc.scalar.dma_start(sw[64:128, :], raw[0:64, :])
                            qc = rp_pool.tile([128, TB], F32, tag="qc")
                            nc.vector.tensor_mul(qc[:], raw[:], cos_sb[:, psl])
                            qs = rp_pool.tile([128, TB], F32, tag="qs")
                            nc.vector.tensor_mul(qs[:], sw[:], ssin_sb[:, psl])
                            nc.vector.tensor_add(
                                dst_sb[:, mt, tsl], qc[:], qs[:])

        # ============ phase B: attention + pipelined output proj ===========
        with ExitStack() as p3:
            wo_pool = p3.enter_context(tc.tile_pool(name="wo", bufs=1))
            wo_sb = wo_pool.tile([128, HPC, D], BF16)
            nc.scalar.dma_start(hmask_sb[:], hmask_d[:])
            for g in range(4):
                nc.scalar.dma_start(
                    wo_sb[:, g, :], woT_d[g * 128:(g + 1) * 128, :])

            ctx_pool = p3.enter_context(tc.tile_pool(name="ctx", bufs=8))
            e_pool = p3.enter_context(tc.tile_pool(name="expt", bufs=4))
            rs_pool = p3.enter_context(tc.tile_pool(name="rsum", bufs=6))
            n_pool = p3.enter_context(tc.tile_pool(name="norm", bufs=2))
            o_pool = p3.enter_context(tc.tile_pool(name="osb", bufs=3))
            s_ps_pool = p3.enter_context(
                tc.tile_pool(name="sps", bufs=2, space="PSUM"))
            a_ps_pool = p3.enter_context(
                tc.tile_pool(name="aps", bufs=1, space="PSUM"))
            r_ps_pool = p3.enter_context(
                tc.tile_pool(name="rps", bufs=1, space="PSUM"))
            o_ps_pool = p3.enter_context(
                tc.tile_pool(name="ops", bufs=2, space="PSUM"))

            def make_units(pctx, pb, pi):
                # 32 deferred out-proj work units for the previous block:
                # each accumulates 4 head slices into one [128,512] PSUM
                # tile and drains it (bf16) into a [128, D] row buffer via
                # the scalar engine; the full row DMAs once per 128 tokens.
                osb_rows = {}

                def unit(tt, et_i):
                    def run():
                        if et_i == 0:
                            osb_rows[tt] = o_pool.tile([128, D], BF16,
                                                       tag="osb",
                                                       name=f"osb{tt}")
                        o_ps = o_ps_pool.tile([128, 512], F32, tag="ops")
                        for h in range(HPC):
                            nc.tensor.matmul(
                                o_ps[:],
                                pctx[h][:, tt * 128:(tt + 1) * 128],
                                wo_sb[:, h, et_i * 512:(et_i + 1) * 512],
                                start=(h == 0), stop=(h == HPC - 1))
                        nc.scalar.copy(
                            osb_rows[tt][:, et_i * 512:(et_i + 1) * 512],
                            o_ps[:])
                        if et_i == 7:
                            row = pb * S + pi * IT + tt * 128
                            nc.sync.dma_start(out_d[row:row + 128, :],
                                              osb_rows[tt][:])
                    return run
                return [unit(tt, e) for tt in range(IT // 128)
                        for e in range(D // 512)]

            prev = None          # (ctx tiles, b, i) of the previous block
            for b in range(B):
                for i in range(NIT):
                    units = make_units(*prev) if prev is not None else []
                    njt = (i + 1) * IT // 128
                    npair = njt // 2
                    # interleave slots: one per score pair, plus one per
                    # head end (covers the ladder/recip/ctx-mul latency)
                    nslots = HPC * (npair + 1)
                    done = 0
                    ctx_i = []
                    for h in range(HPC):
                        qt = q_sb[:, h,
                                  b * S + i * IT: b * S + (i + 1) * IT]
                        ctx_ps = a_ps_pool.tile([128, IT], F32, tag="ctxps")
                        # binary reduction ladder for the denominator:
                        # combine eagerly so at most one live tile per
                        # level (bounded pool pressure, no deadlock)
                        levels = {}

                        def rs_push(t, lvl=0):
                            while lvl in levels:
                                prev_t = levels.pop(lvl)
                                acc = rs_pool.tile([128, IT], BF16,
                                                   tag="ps2")
                                nc.vector.tensor_add(acc[:], prev_t[:],
                                                     t[:])
                                t = acc
                                lvl += 1
                            levels[lvl] = t
                        for p in range(npair):
                            jt0 = 2 * p
                            s_ps = s_ps_pool.tile([128, 1024], F32,
                                                  tag="sps")
                            for half in range(2):
                                jt = jt0 + half
                                nc.tensor.matmul(
                                    s_ps[:, half * 512:(half + 1) * 512],
                                    k_sb[:, h, b * S + jt * 128:
                                         b * S + (jt + 1) * 128],
                                    qt, start=True, stop=True)
                            et = e_pool.tile([128, 2 * IT], BF16,
                                             tag="et")
                            doff0 = jt0 * 128 - i * IT
                            if doff0 + 128 < 0:   # fully off-diagonal
                                nc.scalar.activation(
                                    et[:], s_ps[:], EXPFN, scale=SCALE)
                            else:
                                ef = e_pool.tile([128, 1024], BF16,
                                                 tag="ef", bufs=2)
                                nc.scalar.activation(ef[:], s_ps[:],
                                                     EXPFN, scale=SCALE)
                                for half in range(2):
                                    doff = doff0 + half * 128
                                    nc.vector.tensor_mul(
                                        et[:, half * IT:
                                           (half + 1) * IT],
                                        ef[:, half * 512:
                                           (half + 1) * 512],
                                        hmask_sb[:, 384 - doff:
                                                 896 - doff])
                            for half in range(2):
                                jt = jt0 + half
                                nc.tensor.matmul(
                                    ctx_ps[:],
                                    v_sb[:, b * (S // 128) + jt,
                                         h * 128:(h + 1) * 128],
                                    et[:, half * IT:(half + 1) * IT],
                                    start=(jt == 0),
                                    stop=(jt == njt - 1))
                            psum = rs_pool.tile([128, IT], BF16,
                                                tag="ps2")
                            nc.vector.tensor_add(
                                psum[:], et[:, 0:IT], et[:, IT:2 * IT])
                            rs_push(psum)
                            slot = h * (npair + 1) + p + 1
                            want = (len(units) * slot + nslots - 1) // nslots
                            while done < min(want, len(units)):
                                units[done]()
                                done += 1
                        # fold remaining ladder levels
                        rem = [levels[l] for l in sorted(levels)]
                        total = rem[0]
                        for t in rem[1:]:
                            acc = rs_pool.tile([128, IT], BF16,
                                               tag="ps2")
                            nc.vector.tensor_add(acc[:], total[:], t[:])
                            total = acc
                        rs_ps = r_ps_pool.tile([128, IT], F32, tag="rps")
                        nc.tensor.matmul(rs_ps[:], ones_sb[:], total[:],
                                         start=True, stop=True)
                        slot = (h + 1) * (npair + 1)
                        want = (len(units) * slot + nslots - 1) // nslots
                        while done < min(want, len(units)):
                            units[done]()
                            done += 1
                        recip = n_pool.tile([128, IT], F32, tag="recip")
                        nc.vector.reciprocal_approx_fast(recip[:], rs_ps[:])
                        ctx_h = ctx_pool.tile([128, IT], BF16, tag="ctx")
                        nc.vector.tensor_mul(ctx_h[:], ctx_ps[:], recip[:])
                        ctx_i.append(ctx_h)
                    while done < len(units):
                        units[done]()
                        done += 1
                    prev = (ctx_i, b, i)

            # drain the final block's output projection
            for u in make_units(*prev):
                u()


def _host_prep(x, Wq, Wk, Wv, Wo):
    import ml_dtypes
    bf16 = ml_dtypes.bfloat16

    x = np.asarray(x, dtype=np.float32)
    xT = np.ascontiguousarray(x.reshape(T, D).T.astype(bf16))     # [D, T]

    # per-core column slices of W.T  -> [ncores][D, MS]
    def col_shards(W):
        WT = np.asarray(W, dtype=np.float32).T.reshape(D, NCORES, MS)
        return np.ascontiguousarray(WT.transpose(1, 0, 2).astype(bf16))
    wqT = col_shards(Wq)
    wkT = col_shards(Wk)
    wvT = col_shards(Wv)
    # per-core row slices of Wo.T -> [ncores][MS, D]
    woT = np.ascontiguousarray(
        np.asarray(Wo, dtype=np.float32).T.reshape(NCORES, MS, D).astype(bf16))

    # rope tables in [hd, s] layout
    inv = (1.0 / (10000.0 ** (np.arange(0, HD, 2, dtype=np.float32) / HD))
           ).astype(np.float32)
    t = np.arange(S, dtype=np.float32)
    freqs = np.outer(t, inv).astype(np.float32)                # [S, 64]
    cos = np.cos(freqs).T                                      # [64, S]
    sin = np.sin(freqs).T
    cosT = np.ascontiguousarray(
        np.concatenate([cos, cos], axis=0).astype(bf16))       # [128, S]
    ssinT = np.ascontiguousarray(
        np.concatenate([-sin, sin], axis=0).astype(bf16))

    # causal mask table: hmask[dj, y] = 1 if dj <= y - 384
    dj = np.arange(128)[:, None]
    y = np.arange(896)[None, :]
    hmask = (dj <= y - 384).astype(bf16)

    return xT, wqT, wkT, wvT, woT, cosT, ssinT, hmask


def kernel(x, mask, Wq, Wk, Wv, Wo, _trace=False):
    del mask  # causal mask is hardcoded (tril), matching the reference
    xT, wqT, wkT, wvT, woT, cosT, ssinT, hmask = _host_prep(x, Wq, Wk, Wv, Wo)

    if "nc" not in _compiled:
        _compiled["nc"] = _build()
    nc = _compiled["nc"]

    in_maps = []
    for c in range(NCORES):
        in_maps.append({
            "xT": xT,
            "wqT": wqT[c],
            "wkT": wkT[c],
            "wvT": wvT[c],
            "woT": woT[c],
            "cosT": cosT,
            "ssinT": ssinT,
            "hmask": hmask,
        })

    res = run_bass_kernel_spmd(nc, in_maps, core_ids=list(range(NCORES)),
                               trace=_trace)

    acc = res.results[0]["outp"].astype(np.float64)
    for c in range(1, NCORES):
        acc += res.results[c]["outp"].astype(np.float64)
    out = acc.astype(np.float32).reshape(B, S, D)
    if _trace:
        kernel.last_exec_time_ns = res.exec_time_ns
        kernel.last_results = res
    return out


# revision 15
# speedup vs baseline: 1.1586x; 1.0305x over previous
"""Trainium2 Bass kernel for CustomAttention (dense transformer block).

Full inputs -> full output. Tensor-parallel over heads across 8 NeuronCores:
core c owns heads [4c, 4c+4) i.e. projection columns [512c, 512c+512).
Each core computes q/k/v projections for its heads (RoPE applied on-chip),
causal attention (softmax without max-subtraction; scores bounded ~19), and
a partial output projection over its 512-wide slice of the contraction dim.
The host sums the 8 partials.

All operands are bf16 (1 col/cycle matmul streaming + FWL weight loads);
PSUM accumulation is fp32. q/k/v stay SBUF-resident between phases (no DRAM
spill). DMA is split across the two HW DGE rings: x-tile streams + output
writes on qSync, weight/table bulk loads + rope swaps on qAct. The x tile
pool and projection PSUM pool are shared between the q/k and v projection
phases so the v matmuls start while the last rope tail drains. wv loads are
spread across the q/k loop so rope swap DMAs don't queue behind bulk loads.

Attention exp runs on score PAIRS ([128,1024] from 2 PSUM banks) to halve
activation overhead; the softmax denominator is a vector bf16 add-tree plus
one ones-matmul per query block. The output projection for query block n is
software-pipelined into the attention of block n+1: its matmuls fill tensor
bubbles left by the scores->exp->pv dependency chain, and its PSUM results
drain through the scalar engine (bf16 cast) into [128, 4096] row tiles
written with one DMA per 128 tokens, keeping the vector engine off the
tensor critical path.
"""

import numpy as np

import concourse.bass as bass
import concourse.tile as tile
from concourse import bacc, mybir
from concourse.bass_utils import run_bass_kernel_spmd

F32 = mybir.dt.float32
BF16 = mybir.dt.bfloat16
EXPFN = mybir.ActivationFunctionType.Exp

D = 4096          # model dim
H = 32            # heads (total)
HD = 128          # head dim
NCORES = 8
HPC = H // NCORES  # heads per core = 4
MS = HPC * HD      # per-core projection slice = 512
B = 2
S = 2048
T = B * S         # 4096 tokens
DT = D // 128     # 32 contraction tiles
SCALE = HD ** -0.5

_compiled = {}


def _build():
    nc = bacc.Bacc("TRN2", target_bir_lowering=False, debug=False,
                   num_devices=NCORES)

    xT_d = nc.dram_tensor("xT", [D, T], BF16, kind="ExternalInput").ap()
    wqT_d = nc.dram_tensor("wqT", [D, MS], BF16, kind="ExternalInput").ap()
    wkT_d = nc.dram_tensor("wkT", [D, MS], BF16, kind="ExternalInput").ap()
    wvT_d = nc.dram_tensor("wvT", [D, MS], BF16, kind="ExternalInput").ap()
    woT_d = nc.dram_tensor("woT", [MS, D], BF16, kind="ExternalInput").ap()
    cos_d = nc.dram_tensor("cosT", [HD, S], BF16, kind="ExternalInput").ap()
    ssin_d = nc.dram_tensor("ssinT", [HD, S], BF16, kind="ExternalInput").ap()
    hmask_d = nc.dram_tensor("hmask", [128, 896], BF16, kind="ExternalInput").ap()
    out_d = nc.dram_tensor("outp", [T, D], BF16, kind="ExternalOutput").ap()

    with tile.TileContext(nc) as tc:
        _emit(nc, tc, xT_d, wqT_d, wkT_d, wvT_d, woT_d, cos_d, ssin_d,
              hmask_d, out_d)

    nc.compile()
    return nc


def _emit(nc, tc, xT_d, wqT_d, wkT_d, wvT_d, woT_d, cos_d, ssin_d,
          hmask_d, out_d):
    from contextlib import ExitStack

    TB = 512                 # token block for projections
    NTB = T // TB            # 8
    IT = 512                 # i-tile (query) width in attention
    NIT = S // IT            # 4 per batch
    XC = 4                   # dt per x-load chunk

    def load_w(w_sb, w_d, g):
        sl = slice(g * 4 * 128, (g + 1) * 4 * 128)
        nc.scalar.dma_start(
            w_sb[:, g * 4:(g + 1) * 4, :],
            w_d[sl, :].rearrange("(dt p) m -> p dt m", p=128))

    with ExitStack() as ctx:
        const_pool = ctx.enter_context(tc.tile_pool(name="const", bufs=1))
        hmask_sb = const_pool.tile([128, 896], BF16)
        ones_sb = const_pool.tile([128, 128], BF16)
        nc.vector.memset(ones_sb[:], 1.0)

        # SBUF-resident q/k/v for the whole kernel.
        qk_pool = ctx.enter_context(tc.tile_pool(name="qkres", bufs=1))
        q_sb = qk_pool.tile([128, HPC, T], BF16)
        k_sb = qk_pool.tile([128, HPC, T], BF16)
        v_sb = qk_pool.tile([128, T // 128, MS], BF16)

        # ============ phases A0 (v) and A1 (q/k + rope) ====================
        # v first: its weight pool then frees into the q/k weight pool, and
        # the shared x/PSUM pools let the phases overlap at the boundary.
        with ExitStack() as pa:
            x_pool = pa.enter_context(tc.tile_pool(name="xs", bufs=3))
            pre_pool = pa.enter_context(tc.tile_pool(name="pre", bufs=1))
            ps_pool = pa.enter_context(
                tc.tile_pool(name="psA", bufs=8, space="PSUM"))
            # prefetched during A0 so A1's first matmuls / first rope and
            # phase B's first masked exp never wait on the qAct ring
            cos_sb = pre_pool.tile([HD, S], BF16)
            ssin_sb = pre_pool.tile([HD, S], BF16)
            wq0_sb = pre_pool.tile([128, MS], BF16)
            wk0_sb = pre_pool.tile([128, MS], BF16)

            with ExitStack() as p0:
                wv_pool = p0.enter_context(tc.tile_pool(name="wv", bufs=1))
                wv_sb = wv_pool.tile([128, DT, MS], BF16)
                for g in range(8):
                    load_w(wv_sb, wvT_d, g)
                nc.scalar.dma_start(wq0_sb[:], wqT_d[0:128, :])
                nc.scalar.dma_start(wk0_sb[:], wkT_d[0:128, :])
                nc.scalar.dma_start(cos_sb[:], cos_d[:])
                nc.scalar.dma_start(ssin_sb[:], ssin_d[:])
                nc.scalar.dma_start(hmask_sb[:], hmask_d[:])

                for tb in range(NTB):
                    tsl = bass.ts(tb, TB)
                    pss = [ps_pool.tile([128, MS], F32, tag="ps",
                                        name=f"psA0_{_g}")
                           for _g in range(TB // 128)]
                    for g in range(DT // XC):
                        xc = x_pool.tile([128, XC, TB], BF16, tag="x")
                        nc.sync.dma_start(
                            xc[:],
                            xT_d[g * XC * 128:(g + 1) * XC * 128, tsl]
                            .rearrange("(dt p) t -> p dt t", p=128))
                        for j in range(XC):
                            dt = g * XC + j
                            for tt in range(TB // 128):
                                nc.tensor.matmul(
                                    pss[tt][:],
                                    xc[:, j, tt * 128:(tt + 1) * 128],
                                    wv_sb[:, dt, :],
                                    start=(dt == 0), stop=(dt == DT - 1))
                    for tt in range(TB // 128):
                        nc.vector.tensor_copy(
                            v_sb[:, tb * (TB // 128) + tt, :], pss[tt][:])

            # ---- A1: q/k projections + rope (wv space -> wq/wk) -----------
            with ExitStack() as p1:
                wq_pool = p1.enter_context(tc.tile_pool(name="wqk", bufs=1))
                rp_pool = p1.enter_context(tc.tile_pool(name="rope", bufs=2))

                wq_sb = wq_pool.tile([128, DT, MS], BF16)
                wk_sb = wq_pool.tile([128, DT, MS], BF16)

                # dt=0 came from the prefetch pool; single-dt chunks next
                # for fastest start, then the remaining groups.
                for dt in range(1, 4):
                    nc.scalar.dma_start(
                        wq_sb[:, dt, :], wqT_d[dt * 128:(dt + 1) * 128, :])
                    nc.scalar.dma_start(
                        wk_sb[:, dt, :], wkT_d[dt * 128:(dt + 1) * 128, :])
                for g in range(1, 8):
                    load_w(wq_sb, wqT_d, g)
                    load_w(wk_sb, wkT_d, g)

                for tb in range(NTB):
                    tsl = bass.ts(tb, TB)
                    psl = slice((tb * TB) % S, (tb * TB) % S + TB)
                    # last tb runs k first so the PSUM banks phase B's first
                    # score pair lands on are freed by the earlier rope
                    wlist = ((wq_sb, q_sb), (wk_sb, k_sb))
                    if tb == NTB - 1:
                        wlist = ((wk_sb, k_sb), (wq_sb, q_sb))
                    for w_sb, dst_sb in wlist:
                        pss = [ps_pool.tile([128, TB], F32, tag="ps",
                                            name=f"psA_{_g}")
                               for _g in range(HPC)]
                        for g in range(DT // XC):
                            xc = x_pool.tile([128, XC, TB], BF16, tag="x")
                            nc.sync.dma_start(
                                xc[:],
                                xT_d[g * XC * 128:(g + 1) * XC * 128, tsl]
                                .rearrange("(dt p) t -> p dt t", p=128))
                            for j in range(XC):
                                dt = g * XC + j
                                if dt == 0:
                                    w0 = wq0_sb if w_sb is wq_sb else wk0_sb
                                for mt in range(HPC):
                                    wsl = (w0[:, mt * 128:(mt + 1) * 128]
                                           if dt == 0 else
                                           w_sb[:, dt, mt * 128:(mt + 1) * 128])
                                    nc.tensor.matmul(
                                        pss[mt][:], wsl, xc[:, j, :],
                                        start=(dt == 0), stop=(dt == DT - 1))
                        for mt in range(HPC):
                            raw = rp_pool.tile([128, TB], F32, tag="raw")
                            nc.scalar.copy(raw[:], pss[mt][:])
                            # rotate-half operand: partitions swapped by 64
                            sw = rp_pool.tile([128, TB], F32, tag="sw")
                            nc.scalar.dma_start(sw[0:64, :], raw[64:128, :])
                            nc.scalar.dma_start(sw[64:128, :], raw[0:64, :])
                            qc = rp_pool.tile([128, TB], F32, tag="qc")
                            nc.vector.tensor_mul(qc[:], raw[:], cos_sb[:, psl])
                            qs = rp_pool.tile([128, TB], F32, tag="qs")
                            nc.vector.tensor_mul(qs[:], sw[:], ssin_sb[:, psl])
                            nc.vector.tensor_add(
                                dst_sb[:, mt, tsl], qc[:], qs[:])

        # ============ phase B: attention + pipelined output proj ===========
        with ExitStack() as p3:
            wo_pool = p3.enter_context(tc.tile_pool(name="wo", bufs=1))
            wo_sb = wo_pool.tile([128, HPC, D], BF16)
            # column-chunk order: the first out-proj units need all four
            # head rows of the lowest output columns first
            for c in range(4):
                csl = slice(c * 1024, (c + 1) * 1024)
                nc.scalar.dma_start(
                    wo_sb[:, :, csl],
                    woT_d[:, csl].rearrange("(g p) c -> p g c", p=128))

            ctx_pool = p3.enter_context(tc.tile_pool(name="ctx", bufs=8))
            e_pool = p3.enter_context(tc.tile_pool(name="expt", bufs=4))
            rs_pool = p3.enter_context(tc.tile_pool(name="rsum", bufs=6))
            n_pool = p3.enter_context(tc.tile_pool(name="norm", bufs=2))
            o_pool = p3.enter_context(tc.tile_pool(name="osb", bufs=3))
            s_ps_pool = p3.enter_context(
                tc.tile_pool(name="sps", bufs=2, space="PSUM"))
            a_ps_pool = p3.enter_context(
                tc.tile_pool(name="aps", bufs=1, space="PSUM"))
            r_ps_pool = p3.enter_context(
                tc.tile_pool(name="rps", bufs=1, space="PSUM"))
            o_ps_pool = p3.enter_context(
                tc.tile_pool(name="ops", bufs=2, space="PSUM"))

            def make_units(pctx, pb, pi, fine_dma=False):
                # 32 deferred out-proj work units for the previous block:
                # each accumulates 4 head slices into one [128,512] PSUM
                # tile and drains it (bf16) into a [128, D] row buffer via
                # the scalar engine; the full row DMAs once per 128 tokens.
                # fine_dma (final drain): DMA each 512-slice immediately so
                # the writes overlap the remaining units instead of piling
                # into one serial burst after the last matmul.
                osb_rows = {}

                def unit(tt, et_i):
                    def run():
                        if et_i == 0:
                            osb_rows[tt] = o_pool.tile([128, D], BF16,
                                                       tag="osb",
                                                       name=f"osb{tt}")
                        o_ps = o_ps_pool.tile([128, 512], F32, tag="ops")
                        for h in range(HPC):
                            nc.tensor.matmul(
                                o_ps[:],
                                pctx[h][:, tt * 128:(tt + 1) * 128],
                                wo_sb[:, h, et_i * 512:(et_i + 1) * 512],
                                start=(h == 0), stop=(h == HPC - 1))
                        osl = slice(et_i * 512, (et_i + 1) * 512)
                        nc.scalar.copy(osb_rows[tt][:, osl], o_ps[:])
                        row = pb * S + pi * IT + tt * 128
                        if fine_dma:
                            nc.sync.dma_start(out_d[row:row + 128, osl],
                                              osb_rows[tt][:, osl])
                        elif et_i == 7:
                            nc.sync.dma_start(out_d[row:row + 128, :],
                                              osb_rows[tt][:])
                    return run
                return [unit(tt, e) for tt in range(IT // 128)
                        for e in range(D // 512)]

            prev = None          # (ctx tiles, b, i) of the previous block
            for b in range(B):
                for i in range(NIT):
                    units = make_units(*prev) if prev is not None else []
                    njt = (i + 1) * IT // 128
                    npair = njt // 2
                    # interleave slots: one per score pair, plus one per
                    # head end (covers the ladder/recip/ctx-mul latency)
                    nslots = HPC * (npair + 1)
                    done = 0
                    ctx_i = []
                    for h in range(HPC):
                        qt = q_sb[:, h,
                                  b * S + i * IT: b * S + (i + 1) * IT]
                        ctx_ps = a_ps_pool.tile([128, IT], F32, tag="ctxps")
                        # binary reduction ladder for the denominator:
                        # combine eagerly so at most one live tile per
                        # level (bounded pool pressure, no deadlock)
                        levels = {}

                        def rs_push(t, lvl=0):
                            while lvl in levels:
                                prev_t = levels.pop(lvl)
                                acc = rs_pool.tile([128, IT], BF16,
                                                   tag="ps2")
                                nc.vector.tensor_add(acc[:], prev_t[:],
                                                     t[:])
                                t = acc
                                lvl += 1
                            levels[lvl] = t
                        for p in range(npair):
                            jt0 = 2 * p
                            s_ps = s_ps_pool.tile([128, 1024], F32,
                                                  tag="sps")
                            for half in range(2):
                                jt = jt0 + half
                                nc.tensor.matmul(
                                    s_ps[:, half * 512:(half + 1) * 512],
                                    k_sb[:, h, b * S + jt * 128:
                                         b * S + (jt + 1) * 128],
                                    qt, start=True, stop=True)
                            et = e_pool.tile([128, 2 * IT], BF16,
                                             tag="et")
                            doff0 = jt0 * 128 - i * IT
                            if doff0 + 128 < 0:   # fully off-diagonal
                                nc.scalar.activation(
                                    et[:], s_ps[:], EXPFN, scale=SCALE)
                            else:
                                ef = e_pool.tile([128, 1024], BF16,
                                                 tag="ef", bufs=2)
                                nc.scalar.activation(ef[:], s_ps[:],
                                                     EXPFN, scale=SCALE)
                                for half in range(2):
                                    doff = doff0 + half * 128
                                    nc.vector.tensor_mul(
                                        et[:, half * IT:
                                           (half + 1) * IT],
                                        ef[:, half * 512:
                                           (half + 1) * 512],
                                        hmask_sb[:, 384 - doff:
                                                 896 - doff])
                            for half in range(2):
                                jt = jt0 + half
                                # causal: query cols below the diagonal
                                # k-tile contribute nothing — skip them
                                off = max(0, jt * 128 - i * IT)
                                nc.tensor.matmul(
                                    ctx_ps[:, off:IT],
                                    v_sb[:, b * (S // 128) + jt,
                                         h * 128:(h + 1) * 128],
                                    et[:, half * IT + off:(half + 1) * IT],
                                    start=(jt == 0),
                                    stop=(jt == njt - 1))
                            psum = rs_pool.tile([128, IT], BF16,
                                                tag="ps2")
                            nc.vector.tensor_add(
                                psum[:], et[:, 0:IT], et[:, IT:2 * IT])
                            rs_push(psum)
                            slot = h * (npair + 1) + p + 1
                            want = (len(units) * slot + nslots - 1) // nslots
                            while done < min(want, len(units)):
                                units[done]()
                                done += 1
                        # fold remaining ladder levels
                        rem = [levels[l] for l in sorted(levels)]
                        total = rem[0]
                        for t in rem[1:]:
                            acc = rs_pool.tile([128, IT], BF16,
                                               tag="ps2")
                            nc.vector.tensor_add(acc[:], total[:], t[:])
                            total = acc
                        rs_ps = r_ps_pool.tile([128, IT], F32, tag="rps")
                        nc.tensor.matmul(rs_ps[:], ones_sb[:], total[:],
                                         start=True, stop=True)
                        slot = (h + 1) * (npair + 1)
                        want = (len(units) * slot + nslots - 1) // nslots
                        while done < min(want, len(units)):
                            units[done]()
                            done += 1
                        recip = n_pool.tile([128, IT], F32, tag="recip")
                        nc.vector.reciprocal_approx_fast(recip[:], rs_ps[:])
                        ctx_h = ctx_pool.tile([128, IT], BF16, tag="ctx")
                        nc.vector.tensor_mul(ctx_h[:], ctx_ps[:], recip[:])
                        ctx_i.append(ctx_h)
                    while done < len(units):
                        units[done]()
                        done += 1
                    prev = (ctx_i, b, i)

            # drain the final block's output projection
            for u in make_units(*prev, fine_dma=True):
                u()


def _host_prep(x, Wq, Wk, Wv, Wo):
    import ml_dtypes
    bf16 = ml_dtypes.bfloat16

    x = np.asarray(x, dtype=np.float32)
    xT = np.ascontiguousarray(x.reshape(T, D).T.astype(bf16))     # [D, T]

    # per-core column slices of W.T  -> [ncores][D, MS]
    def col_shards(W):
        WT = np.asarray(W, dtype=np.float32).T.reshape(D, NCORES, MS)
        return np.ascontiguousarray(WT.transpose(1, 0, 2).astype(bf16))
    wqT = col_shards(Wq)
    wkT = col_shards(Wk)
    wvT = col_shards(Wv)
    # per-core row slices of Wo.T -> [ncores][MS, D]
    woT = np.ascontiguousarray(
        np.asarray(Wo, dtype=np.float32).T.reshape(NCORES, MS, D).astype(bf16))

    # rope tables in [hd, s] layout
    inv = (1.0 / (10000.0 ** (np.arange(0, HD, 2, dtype=np.float32) / HD))
           ).astype(np.float32)
    t = np.arange(S, dtype=np.float32)
    freqs = np.outer(t, inv).astype(np.float32)                # [S, 64]
    cos = np.cos(freqs).T                                      # [64, S]
    sin = np.sin(freqs).T
    cosT = np.ascontiguousarray(
        np.concatenate([cos, cos], axis=0).astype(bf16))       # [128, S]
    ssinT = np.ascontiguousarray(
        np.concatenate([-sin, sin], axis=0).astype(bf16))

    # causal mask table: hmask[dj, y] = 1 if dj <= y - 384
    dj = np.arange(128)[:, None]
    y = np.arange(896)[None, :]
    hmask = (dj <= y - 384).astype(bf16)

    return xT, wqT, wkT, wvT, woT, cosT, ssinT, hmask


def kernel(x, mask, Wq, Wk, Wv, Wo, _trace=False):
    del mask  # causal mask is hardcoded (tril), matching the reference
    xT, wqT, wkT, wvT, woT, cosT, ssinT, hmask = _host_prep(x, Wq, Wk, Wv, Wo)

    if "nc" not in _compiled:
        _compiled["nc"] = _build()
    nc = _compiled["nc"]

    in_maps = []
    for c in range(NCORES):
        in_maps.append({
            "xT": xT,
            "wqT": wqT[c],
            "wkT": wkT[c],
            "wvT": wvT[c],
            "woT": woT[c],
            "cosT": cosT,
            "ssinT": ssinT,
            "hmask": hmask,
        })

    res = run_bass_kernel_spmd(nc, in_maps, core_ids=list(range(NCORES)),
                               trace=_trace)

    acc = res.results[0]["outp"].astype(np.float64)
    for c in range(1, NCORES):
        acc += res.results[c]["outp"].astype(np.float64)
    out = acc.astype(np.float32).reshape(B, S, D)
    if _trace:
        kernel.last_exec_time_ns = res.exec_time_ns
        kernel.last_results = res
    return out
